# revision 13
# baseline (speedup 1.0000x reference)
"""3-layer GCN (PyG-style GCNConv with self-loops + symmetric norm) on 8
Trainium2 NeuronCores.

Distribution (1D graph partitioning):
  - nodes split into 8 contiguous blocks of 6250 rows, one per core
  - edges partitioned by destination core, sorted by destination node
  - 256x256 weights replicated on every core

Per layer, per core (fp8-e3m4 message tables, fp32 accumulation):
  1. GEMM: ytilde_c = (s_l*dinv) * (h_c @ W.T) cast to f8e3 (PE transpose of
     h tiles, 2 accumulating fp16 matmuls, dinv*scale fold + f8 cast on the
     Scalar engine), staged into per-group local DRAM tiles
  2. Row-grouped AllGather: the per-core node rows are split into NGROUP
     groups; group g's AllGather fires as soon as its GEMM chunks finish, so
     all but the last AG overlap the preceding message-passing loop. Each AG
     writes its own Shared table T_g [8*rows_g, 256] (rank-major rows), which
     is the gather table for that group's edges directly. The bias row is the
     last row of the last group (rank 0's copy is referenced).
  3. message passing for the core's ~106k incoming edges (incl self-loops):
     - edges sorted by dst, grouped into 128-node dst chunks; within a chunk
       edges are bucketed by the src node's GROUP, and each (chunk, group)
       bucket is packed into 128-edge tiles (so each tile gathers from one
       T_g); G tiles per dma_gather instruction, int16 row indices
     - selection matrix selT[e, d] = (dst_local[e] == d) built on-chip with
       one DVE is_equal per gather group (f16); PSUM-accumulated matmuls
       out_chunk += selT.T @ msg with f8e3 moving operand
     - bias enters as a reserved edge (slot 0 of each chunk's group-2 bucket)
       whose selection column is sqrt(deg[dst]) and whose gathered row is
       s_l*bias
  4. epilogue: relu((dinv/s_l) * psum), residual add (layers 1,2), h update
"""

import math
import os

import numpy as np

import concourse.bass as bass
import concourse.mybir as mybir
import concourse.tile as tile
from concourse import bacc
from concourse.bass_utils import run_bass_kernel_spmd
from concourse.masks import make_identity

F32 = mybir.dt.float32
F16 = mybir.dt.float16
F8 = mybir.dt.float8e3
I16 = mybir.dt.int16
I32 = mybir.dt.int32

N_NODES = 50000
HID = 256
NCORES = 8
NPC = N_NODES // NCORES          # 6250 nodes per core
NCHUNK = math.ceil(NPC / 128)    # 49 dst chunks per core
G = 8                            # edge tiles per gather instruction
                                 # (G=16 → 2048-idx gathers hang on HW:
                                 # descriptor ring overflow)
PAD_DST = 255.0                  # dst_local sentinel that matches no iota lane
NLAYERS = 3
NSWDGE_QUEUES = 4                # parallel SWDGE descriptor-gen queues
MM_DT = F16                      # eq/weight dtype (PSUM accum is f32)
TB_DT = F8                       # message table dtype
MW = G * 4 + G // 2              # meta int32 words: idx int16 x8G + dst f16 xG

NGROUP = 2                       # row-grouped AllGathers per layer
GCHUNKS = (25, 24)               # dst chunks per AG group (sums to 49)
SCALES = (4.0, 16.0, 16.0)       # per-layer table scale (fits e3m4 range)

GSTART = np.concatenate([[0], np.cumsum(GCHUNKS)]).astype(int)  # chunk starts
# rows per group per core (last group gets the partial chunk + bias row)
GROWS = [
    min(NPC, int(GSTART[g + 1]) * 128) - int(GSTART[g]) * 128
    for g in range(NGROUP)
]
GROWS[-1] += 1  # bias row
assert max(GROWS) * NCORES <= 32767, "gather indices must fit int16"

BIAS_ROW = GROWS[-1] - 1  # rank 0's bias row within T_{NGROUP-1}

_cache = {}


def _group_of_local(r):
    return np.minimum(
        np.searchsorted(GSTART[1:] * 128, r, side="right"), NGROUP - 1
    )


def _pack_stream(flat_idx, flat_dst, NG):
    """flat_* are [NG*G*128] slot arrays in (tile, slot) order.

    Returns packed meta [NG*128, MW] int32: per row
    [G*8 int16 idx | G f16 dst].
    """
    dstT = (
        flat_dst.reshape(NG, G, 128).transpose(0, 2, 1).reshape(NG * 128, G)
    )
    idxT = np.zeros((NG * 128, G * 8), dtype=np.int16)
    vals = flat_idx.reshape(NG, G * 128)
    for g in range(NG):
        a16 = vals[g].reshape(G * 8, 16).T  # [16, G*8]; slot i at [i%16, i//16]
        idxT[g * 128 : (g + 1) * 128] = np.tile(a16, (8, 1))
    meta = np.zeros((NG * 128, MW), dtype=np.int32)
    meta[:, : G * 4] = idxT.view(np.int32)
    meta[:, G * 4 :] = dstT.astype(np.float16).reshape(-1, G).view(np.int32)
    return meta


def _preprocess(edge_index):
    """Edge partitioning by destination + per-core per-group stream layouts."""
    src = np.asarray(edge_index[0], dtype=np.int64)
    dst = np.asarray(edge_index[1], dtype=np.int64)
    deg = (np.bincount(dst, minlength=N_NODES) + 1).astype(np.float32)
    dinv = (1.0 / np.sqrt(deg)).astype(np.float32)

    loops = np.arange(N_NODES, dtype=np.int64)
    s = np.concatenate([src, loops])
    d = np.concatenate([dst, loops])
    # edges[core][chunk][group] -> (idx_in_T_g, dst_local)
    edges = []
    cnt = np.zeros((NCORES, NCHUNK, NGROUP), dtype=np.int64)
    for c in range(NCORES):
        lo = c * NPC
        m = (d >= lo) & (d < lo + NPC)
        cs, cd = s[m], (d[m] - lo)
        order = np.argsort(cd, kind="stable")
        cs, cd = cs[order], cd[order]
        bounds = np.searchsorted(cd, np.arange(0, NCHUNK + 1) * 128)
        rows = []
        for ch in range(NCHUNK):
            a, b = bounds[ch], bounds[ch + 1]
            es, ed = cs[a:b], cd[a:b] - ch * 128
            sc, sr = np.divmod(es, NPC)          # src core, src local row
            grp = _group_of_local(sr)
            lr = sr - GSTART[grp] * 128
            row = sc * np.asarray(GROWS)[grp] + lr  # row within T_grp
            by_g = []
            for g in range(NGROUP):
                mk = grp == g
                by_g.append((row[mk], ed[mk]))
                cnt[c, ch, g] = mk.sum() + (1 if g == NGROUP - 1 else 0)
            rows.append(by_g)
        edges.append(rows)

    # per-group per-chunk tile counts (max over cores, so SPMD shapes match)
    T = [
        [int(np.ceil(cnt[:, ch, g].max() / 128)) for ch in range(NCHUNK)]
        for g in range(NGROUP)
    ]
    tiles = [int(np.sum(T[g])) for g in range(NGROUP)]
    NG = [math.ceil(t / G) for t in tiles]
    start = [
        np.concatenate([[0], np.cumsum(T[g])]).astype(int) for g in range(NGROUP)
    ]

    per_core = []
    for c in range(NCORES):
        mets = []
        for g in range(NGROUP):
            fi = np.zeros(NG[g] * G * 128, dtype=np.int64)  # pad idx: row 0
            fd = np.full(NG[g] * G * 128, PAD_DST, dtype=np.float32)
            for ch in range(NCHUNK):
                ris, eds = edges[c][ch][g]
                p0 = start[g][ch] * 128
                if g == NGROUP - 1:  # bias edge first
                    L = len(ris) + 1
                    fi[p0 : p0 + L] = np.concatenate([[BIAS_ROW], ris])
                    fd[p0 + 1 : p0 + L] = eds
                else:
                    L = len(ris)
                    fi[p0 : p0 + L] = ris
                    fd[p0 : p0 + L] = eds
            mets.append(_pack_stream(fi, fd, NG[g]))
        # dinv columns: [128, NCHUNK, 2*nlayers] (cast scale s_l*dinv,
        # epilogue scale dinv/s_l), chunk-column layout
        dv = np.zeros(NCHUNK * 128, dtype=np.float32)
        dv[:NPC] = dinv[c * NPC : (c + 1) * NPC]
        dcol = dv.reshape(NCHUNK, 128).T  # [128, NCHUNK]
        dvs = np.zeros((128, NCHUNK, 2 * NLAYERS), dtype=np.float32)
        for l, s_l in enumerate(SCALES):
            dvs[:, :, 2 * l] = dcol * s_l
            dvs[:, :, 2 * l + 1] = dcol / s_l
        # sqrt(deg) rows (bias column values), [1, NCHUNK*128]
        sq = np.zeros(NCHUNK * 128, dtype=np.float16)
        sq[:NPC] = np.sqrt(deg[c * NPC : (c + 1) * NPC]).astype(np.float16)
        per_core.append(
            (mets, np.ascontiguousarray(dvs.reshape(128, -1)), sq.reshape(1, -1))
        )

    sched = (
        tuple(tuple(T[g]) for g in range(NGROUP)),
        tuple(tiles),
        tuple(NG),
    )
    return sched, per_core


def _build(sched, nlayers=3):
    T, tiles, NG = sched
    nc = bacc.Bacc(
        "TRN2",
        target_bir_lowering=False,
        debug=False,
        num_devices=NCORES,
        num_swdge_queues=NSWDGE_QUEUES,
    )
    x_ap = nc.dram_tensor("x", [NPC, HID], F32, kind="ExternalInput").ap()
    wts = nc.dram_tensor(
        "wts", [2 * nlayers, 128, HID], MM_DT, kind="ExternalInput"
    ).ap()
    bias = nc.dram_tensor("bias", [nlayers, HID], TB_DT, kind="ExternalInput").ap()
    consts = nc.dram_tensor("consts", [128, 64], I32, kind="ExternalInput").ap()
    dinvc = nc.dram_tensor(
        "dinvc", [128, NCHUNK * 2 * nlayers], F32, kind="ExternalInput"
    ).ap()
    sqdeg = nc.dram_tensor(
        "sqdeg", [1, NCHUNK * 64], I32, kind="ExternalInput"
    ).ap()
    mets = [
        nc.dram_tensor(f"met{g}", [NG[g] * 128, MW], I32, kind="ExternalInput").ap()
        for g in range(NGROUP)
    ]
    out_ap = nc.dram_tensor("out", [NPC, HID], F32, kind="ExternalOutput").ap()

    with tile.TileContext(nc) as tc:
        with tc.tile_pool(name="const", bufs=1) as cpool, \
             tc.tile_pool(name="hpool", bufs=1) as hpool, \
             tc.tile_pool(name="work", bufs=3) as work, \
             tc.tile_pool(name="meta", bufs=12) as meta, \
             tc.tile_pool(name="msgp", bufs=12) as msgp, \
             tc.tile_pool(name="eqp", bufs=12) as eqp, \
             tc.tile_pool(name="ptp", bufs=2, space="PSUM") as ptp, \
             tc.tile_pool(name="ypp", bufs=2, space="PSUM") as ypp, \
             tc.tile_pool(name="psp", bufs=4, space="PSUM") as psp, \
             tc.tile_pool(name="dram", bufs=1, space="DRAM") as dram:

            identity = cpool.tile([128, 128], F32)
            make_identity(nc, identity[:])
            cst = cpool.tile([128, 64], I32)
            nc.sync.dma_start(out=cst[:], in_=consts[:])
            iota_sb = cst[:, 0:64].bitcast(F16)    # [128,128] rows = 0..127
            dv_sb = cpool.tile([128, NCHUNK * 2 * nlayers], F32)
            nc.sync.dma_start(out=dv_sb[:], in_=dinvc[:])
            dvv = dv_sb[:].rearrange("p (c l) -> p c l", c=NCHUNK)
            sq_sb = cpool.tile([1, NCHUNK * 64], I32)
            nc.sync.dma_start(out=sq_sb[:], in_=sqdeg[:])
            sq16 = sq_sb.bitcast(F16)  # [1, NCHUNK*128] f16

            wt_sb = cpool.tile([128, 2 * nlayers * HID], MM_DT)
            for i in range(2 * nlayers):
                nc.sync.dma_start(
                    out=wt_sb[:, i * HID : (i + 1) * HID], in_=wts[i]
                )

            # h lives in SBUF, one tile per 128-node chunk, updated in place
            h_sb = [
                hpool.tile([128, HID], F32, tag=f"h{c}", name=f"h_sb{c}")
                for c in range(NCHUNK)
            ]
            for c in range(NCHUNK):
                rows = min(128, NPC - c * 128)
                nc.sync.dma_start(
                    out=h_sb[c][:rows], in_=x_ap[c * 128 : c * 128 + rows, :]
                )

            # per-layer, per-group AG input tiles + Shared gather tables
            y_cs = [
                [
                    dram.tile([GROWS[g], HID], TB_DT, name=f"y_c{l}_{g}")
                    for g in range(NGROUP)
                ]
                for l in range(nlayers)
            ]
            y_tbl = [
                [
                    dram.tile(
                        [NCORES * GROWS[g], HID],
                        TB_DT,
                        addr_space="Shared",
                        name=f"y_tbl{l}_{g}",
                    )
                    for g in range(NGROUP)
                ]
                for l in range(nlayers)
            ]
            for l in range(nlayers):
                nc.sync.dma_start(
                    out=y_cs[l][NGROUP - 1][BIAS_ROW : BIAS_ROW + 1, :],
                    in_=bias[l : l + 1, :],
                )

            def gemm_chunk(l, c):
                """y rows of chunk c = (s_l*dinv) * (h_sb[c] @ W_l.T)"""
                rows = min(128, NPC - c * 128)
                grp = int(_group_of_local(np.asarray(c * 128)))
                gc = c - int(GSTART[grp])
                hT = work.tile([128, HID], MM_DT, tag="hT", name="hT")
                for k in range(2):
                    pt = ptp.tile([128, 128], F32, tag="pt", name="pt")
                    nc.tensor.transpose(
                        out=pt[:, :rows],
                        in_=h_sb[c][:rows, k * 128 : (k + 1) * 128],
                        identity=identity[:rows, :rows],
                    )
                    # DVE, not Scalar: the Scalar engine is the GEMM
                    # pipeline's serial resource (cast epilogue)
                    nc.vector.tensor_scalar_add(
                        hT[:, k * 128 : k * 128 + rows], pt[:, :rows], 0.0
                    )
                yp = ypp.tile([128, HID], F32, tag="yp", name="yp")
                for k in range(2):
                    nc.tensor.matmul(
                        out=yp[:rows, :],
                        lhsT=hT[:, k * 128 : k * 128 + rows],
                        rhs=wt_sb[:, (2 * l + k) * HID : (2 * l + k + 1) * HID],
                        start=(k == 0),
                        stop=(k == 1),
                    )
                y_sb = work.tile([128, HID], TB_DT, tag="y_sb", name="y_sb")
                nc.scalar.activation(
                    out=y_sb[:rows],
                    in_=yp[:rows, :],
                    func=mybir.ActivationFunctionType.Identity,
                    scale=dvv[:rows, c, 2 * l : 2 * l + 1],
                )
                nc.sync.dma_start(
                    out=y_cs[l][grp][gc * 128 : gc * 128 + rows, :],
                    in_=y_sb[:rows],
                )

            def fire_ag(l, grp):
                nc.gpsimd.collective_compute(
                    "AllGather",
                    mybir.AluOpType.bypass,
                    replica_groups=[list(range(NCORES))],
                    ins=[y_cs[l][grp][:].opt()],
                    outs=[y_tbl[l][grp][:].opt()],
                )

            ag_fire_chunks = {int(GSTART[g + 1]) - 1: g for g in range(NGROUP)}

            for c in range(NCHUNK):
                gemm_chunk(0, c)
                if c in ag_fire_chunks:
                    fire_ag(0, ag_fire_chunks[c])

            for l in range(nlayers):
                pos = [0] * NGROUP
                bufs = {}
                for ci in range(NCHUNK):
                    crows = min(128, NPC - ci * 128)
                    ntot = sum(T[g][ci] for g in range(NGROUP))
                    ps = psp.tile([128, HID], F32, tag="ps", name="ps")
                    jj = 0
                    for sg in range(NGROUP):
                        for t in range(T[sg][ci]):
                            st = pos[sg]
                            g, col = divmod(st, G)
                            if col == 0:
                                rem = min(G, tiles[sg] - g * G)
                                met_sb = meta.tile(
                                    [128, MW], I32, tag="met_sb", name="met_sb"
                                )
                                nc.scalar.dma_start(
                                    out=met_sb[:],
                                    in_=mets[sg][g * 128 : (g + 1) * 128, :],
                                )
                                idx_sb = met_sb[:, : G * 4].bitcast(I16)
                                dst_sb = met_sb[:, G * 4 :].bitcast(F16)
                                msg = msgp.tile(
                                    [128, G * HID], TB_DT, tag="msg", name="msg"
                                )
                                nc.gpsimd.dma_gather(
                                    out_ap=msg[:, : rem * HID].rearrange(
                                        "p (g d) -> p g d", g=rem
                                    ),
                                    in_ap=y_tbl[l][sg][:, :],
                                    idxs_ap=idx_sb[:, : rem * 8],
                                    num_idxs=rem * 128,
                                    num_idxs_reg=rem * 128,
                                    elem_size=HID,
                                    queue_num=(g + sg) % NSWDGE_QUEUES,
                                )
                                eq = eqp.tile(
                                    [128, G * 128], MM_DT, tag="eq", name="eq"
                                )
                                eq3 = eq[:, : rem * 128].rearrange(
                                    "p (g d) -> p g d", g=rem
                                )
                                nc.vector.tensor_tensor(
                                    out=eq3,
                                    in0=dst_sb[:, :rem, None].to_broadcast(
                                        (128, rem, 128)
                                    ),
                                    in1=iota_sb[:, None, :].to_broadcast(
                                        (128, rem, 128)
                                    ),
                                    op=mybir.AluOpType.is_equal,
                                )
                                bufs[sg] = (msg, eq)
                            msg, eq = bufs[sg]
                            if sg == NGROUP - 1 and t == 0:
                                # bias edge: its sel column = sqrt(deg[dst])
                                nc.vector.tensor_tensor(
                                    out=eq[0:1, col * 128 : col * 128 + crows],
                                    in0=eq[0:1, col * 128 : col * 128 + crows],
                                    in1=sq16[0:1, ci * 128 : ci * 128 + crows],
                                    op=mybir.AluOpType.add,
                                )
                            nc.tensor.matmul(
                                out=ps[:, :],
                                lhsT=eq[:, col * 128 : (col + 1) * 128],
                                rhs=msg[:, col * HID : (col + 1) * HID],
                                start=(jj == 0),
                                stop=(jj == ntot - 1),
                            )
                            pos[sg] += 1
                            jj += 1
                    # epilogue: relu((dinv/s_l) * psum), residual, h update
                    if l == 0:
                        nc.scalar.activation(
                            out=h_sb[ci][:crows],
                            in_=ps[:crows, :],
                            func=mybir.ActivationFunctionType.Relu,
                            scale=dvv[:crows, ci, 2 * l + 1 : 2 * l + 2],
                        )
                    else:
                        o_sb = work.tile([128, HID], F32, tag="o_sb", name="o_sb")
                        nc.scalar.activation(
                            out=o_sb[:crows],
                            in_=ps[:crows, :],
                            func=mybir.ActivationFunctionType.Relu,
                            scale=dvv[:crows, ci, 2 * l + 1 : 2 * l + 2],
                        )
                        if l < nlayers - 1:
                            nc.vector.tensor_add(
                                out=h_sb[ci][:crows],
                                in0=o_sb[:crows],
                                in1=h_sb[ci][:crows],
                            )
                        else:
                            nc.vector.tensor_add(
                                out=o_sb[:crows],
                                in0=o_sb[:crows],
                                in1=h_sb[ci][:crows],
                            )
                            nc.sync.dma_start(
                                out=out_ap[ci * 128 : ci * 128 + crows, :],
                                in_=o_sb[:crows],
                            )
                    if l + 1 < nlayers:
                        gemm_chunk(l + 1, ci)
                # AGs batched after the gather stream: firing them mid-loop
                # would block the in-order GpSimd sequencer (and so all later
                # gather descgen) on the AG's input sems. Here the sems are
                # already satisfied when dispatch reaches them, and the next
                # layer's group-g gathers only wait on their own table, so
                # descgen of group-0 gathers overlaps AG groups 1-2.
                if l + 1 < nlayers:
                    for g in range(NGROUP):
                        fire_ag(l + 1, g)

    nc.compile()
    return nc


def _consts_array():
    iota = np.tile(np.arange(128, dtype=np.float16)[None, :], (128, 1))
    return iota.view(np.int32)  # [128, 64] i32


def kernel(x, edge_index, W0, b0, W1, b1, W2, b2):
    import ml_dtypes

    x = np.asarray(x, dtype=np.float32)
    edge_index = np.asarray(edge_index)
    Ws = [np.asarray(w, dtype=np.float32) for w in (W0, W1, W2)]
    bs = [np.asarray(b, dtype=np.float32) for b in (b0, b1, b2)]

    sched, per_core = _preprocess(edge_index)

    key = (sched, NLAYERS)
    if key not in _cache:
        _cache[key] = _build(sched, nlayers=NLAYERS)
    nc = _cache[key]

    wts = np.stack(
        [w.T[k * 128 : (k + 1) * 128, :] for w in Ws for k in range(2)]
    ).astype(np.float16)
    bias_arr = np.stack(
        [s_l * b for s_l, b in zip(SCALES, bs)]
    ).astype(ml_dtypes.float8_e3m4)
    consts = _consts_array()

    in_maps = []
    for c in range(NCORES):
        met_list, dvs, sq = per_core[c]
        im = {
            "x": np.ascontiguousarray(x[c * NPC : (c + 1) * NPC]),
            "wts": wts,
            "bias": bias_arr,
            "consts": consts,
            "dinvc": dvs,
            "sqdeg": np.ascontiguousarray(sq).view(np.int32),
        }
        for g in range(NGROUP):
            im[f"met{g}"] = met_list[g]
        in_maps.append(im)

    trace = bool(int(os.environ.get("GCN_TRACE", "0")))
    res = run_bass_kernel_spmd(
        nc, in_maps, core_ids=list(range(NCORES)), trace=trace
    )
    if trace:
        kernel.last_exec_time_ns = res.exec_time_ns
        kernel.last_results = res
    out = np.concatenate([res.results[c]["out"] for c in range(NCORES)], axis=0)
    return out
